# revision 8
# baseline (speedup 1.0000x reference)
"""Trainium2 Bass kernel for nn_CameraOptimizer — f16 + PE-diag-matmul design.

Math per camera n (host-side, f64): 12 projective coefficients c0..c11 s.t.
    u = (c0 x0 + c1 x1 + c2 x2 + c3) / z,   z = c8 x0 + c9 x1 + c10 x2 + c11
    v = (c4 x0 + c5 x1 + c6 x2 + c7) / z

Device pipeline per round (camera-sorted slot layout, 8 rounds x 128 slots):
    pun = c0 x0 + c1 x1 + c2 x2          PE: 3 diag matmuls (f16) -> PSUM
    pvn = c4 x0 + c5 x1 + c6 x2          PE: 3 diag matmuls -> PSUM
    pz  = (c10 x2 + c11)                 ACT activation writes PSUM init...
        + c8 x0 + c9 x1                  ...PE matmuls accumulate (start=False)
    rz  = 1/pz                           DVE reciprocal (psum -> f16)
    u   = (pun + c3) * rz                DVE scalar_tensor_tensor -> f16
    vn' = pvn + c7                       ACT activation -> f16
    v   = vn' * rz                       DVE f16 tensor_tensor (2x mode)

The 8 diag weight matrices per round are built by ONE Pool affine_select
([P, 8, 128] 3-dim pattern) from the f32 coef tile; Pool runs a pure diag
stream so its in-order queue never serializes the pipeline. Matmul groups
split at column 512 (PSUM bank limit); the z accumulate-init requires one
PSUM tile per accumulation group (pza/pzb) — two groups on one tile compute
garbage on hardware. PE warmup matmuls hold the p-state ramp (idle gaps
reset it to half speed). One merged 3-plane input DMA and one merged
2-plane output DMA per round keep HWDGE occupancy low; all point data
moves as f16 (halves HBM traffic; rel err ~3e-4 vs the 2e-2 gate).

Sharding: data-parallel over points across the 8 cores; the small
per-camera tables are folded host-side (f64) into the 12 coefficients and
replicated. Host packs points camera-grouped per round (scatter), device
computes, host unscatters.
"""

import numpy as np

import concourse.bass as bass
import concourse.mybir as mybir
import concourse.tile as tile
from concourse.bass_utils import run_bass_kernel_spmd

NUM_CAMERAS = 1000
NCORES = 8
ROUNDS = 8
P = 128
P_LIST = [P] * (ROUNDS - 1) + [NUM_CAMERAS - (ROUNDS - 1) * P]

F32 = mybir.dt.float32
F16 = mybir.dt.float16

LAST_RESULTS = None
LAST_S_LIST = None

BEST_CONFIG = dict(u_act_rounds=(), xbufs=4, warm_mms=8, obufs=4,
                   sbufs=4, head_halves=0, tail_halves=0, q_ahead=True,
                   coef_ring="sync")


def _legalize_single_wait(nc):
    """Hoist extra sync waits onto same-engine NOPs (walrus allows one)."""
    f = nc.m.functions[0]
    for bb in f.blocks:
        out = []
        for inst in list(bb.instructions):
            si = inst.sync_info
            waits = list(si.on_wait) if (si is not None and si.on_wait) else []
            if len(waits) > 1:
                builder = nc.engines[inst.engine]
                for w in waits[:-1]:
                    nop_inst = builder.nop(nofuse=True).ins
                    for bb2 in f.blocks:
                        if bb2.instructions and bb2.instructions[-1] is nop_inst:
                            bb2.instructions.pop()
                            break
                    nop_inst.sync_info = mybir.SyncInfo(on_wait=[w], on_update=[])
                    out.append(nop_inst)
                si.on_wait = [waits[-1]]
            out.append(inst)
        bb.instructions[:] = out


def _camera_coeffs(noisy_K, noisy_R, noisy_t, intrinsic_deltas, rotation_deltas,
                   translation_deltas):
    """[N,12] float32 projective coefficients per camera (computed in f64)."""
    w = rotation_deltas.astype(np.float64)
    theta = np.linalg.norm(w, axis=-1, keepdims=True)
    k = w / np.maximum(theta, 1e-8)
    kx, ky, kz = k[:, 0], k[:, 1], k[:, 2]
    zero = np.zeros_like(kx)
    K = np.stack(
        [zero, -kz, ky, kz, zero, -kx, -ky, kx, zero], axis=-1
    ).reshape(-1, 3, 3)
    I = np.eye(3, dtype=np.float64)
    s = np.sin(theta)[..., None]
    c = np.cos(theta)[..., None]
    R_delta = I + s * K + (1.0 - c) * (K @ K)
    R = R_delta @ noisy_R.astype(np.float64)
    t = (noisy_t + translation_deltas).astype(np.float64)
    Kc = (noisy_K + intrinsic_deltas).astype(np.float64)
    fx, fy, cx, cy = Kc[:, 0], Kc[:, 1], Kc[:, 2], Kc[:, 3]

    n = R.shape[0]
    C = np.empty((n, 12), np.float64)
    C[:, 0:3] = fx[:, None] * R[:, 0, :] + cx[:, None] * R[:, 2, :]
    C[:, 3] = fx * t[:, 0] + cx * t[:, 2]
    C[:, 4:7] = fy[:, None] * R[:, 1, :] + cy[:, None] * R[:, 2, :]
    C[:, 7] = fy * t[:, 1] + cy * t[:, 2]
    C[:, 8:11] = R[:, 2, :]
    C[:, 11] = t[:, 2]
    return C.astype(np.float32)


def _build_program(S_list, xbufs=4, obufs=3, psum_bufs=(1, 1, 2),
                   diag_engine="gpsimd", v_engine="dve_tt", out_engine="sync",
                   warm_mms=0, mm_order="zvu", sbufs=2, dbufs=8,
                   diag_ahead=8, u_act_rounds=(), tail_split=True,
                   coef_split=True, coef_ring="scalar", split_z0=False,
                   head_halves=1, tail_halves=1, q_ahead=True):
    """One-core Bass/Tile program, SPMD across the 8 cores.

    S_list: per-round column widths (points per camera slot, padded).
    psum_bufs: bufs for the (pun, pvn, pzz) PSUM tags; each tile is
        2 banks when max(S) > 512, so the total must fit 8 banks.
    """
    totx = sum(3 * pr * S for pr, S in zip(P_LIST, S_list))
    toto = sum(2 * pr * S for pr, S in zip(P_LIST, S_list))
    mult = mybir.AluOpType.mult
    add = mybir.AluOpType.add
    ident = mybir.ActivationFunctionType.Identity

    nc = bass.Bass()
    xp = nc.dram_tensor("xp", [totx], F16, kind="ExternalInput")
    coef = nc.dram_tensor("coef", [ROUNDS * P, 16], F32, kind="ExternalInput")
    op = nc.dram_tensor("op", [toto], F16, kind="ExternalOutput")

    xbases, obases = [], []
    xoff = ooff = 0
    for r in range(ROUNDS):
        xbases.append(xoff)
        obases.append(ooff)
        xoff += 3 * P_LIST[r] * S_list[r]
        ooff += 2 * P_LIST[r] * S_list[r]

    with tile.TileContext(nc) as tc:
        with (
            tc.tile_pool(name="cpool", bufs=1) as cpool,
            tc.tile_pool(name="dpool", bufs=dbufs) as dpool,
            tc.tile_pool(name="xpool", bufs=xbufs) as xpool,
            tc.tile_pool(name="spool", bufs=sbufs) as spool,
            tc.tile_pool(name="opool", bufs=obufs) as opool,
            tc.tile_pool(name="ppool", bufs=1, space="PSUM") as ppool,
        ):
            # coef DMA: a tiny round-0-only slice first (unblocks the
            # first diag build ~2us earlier), then the rest.
            ctile = cpool.tile([P, ROUNDS * 16], F32)
            cview = coef.ap().rearrange("(r p) c -> p r c", p=P)
            ctv = ctile[:].rearrange("p (r c) -> p r c", r=ROUNDS)
            ceng = getattr(nc, coef_ring)
            if coef_split:
                # tiny round-0 slice wins the first DMA slot on the input
                # ring; the bulk rides the ACT ring in parallel
                ceng.dma_start(out=ctv[:, 0:1], in_=cview[:, 0:1])
                nc.scalar.dma_start(out=ctv[:, 1:ROUNDS],
                                    in_=cview[:, 1:ROUNDS])
            else:
                ceng.dma_start(out=ctv, in_=cview)

            # optional PE warmup: ramps the p-state before round 0's matmuls.
            # Shares the pzz psum tag so no extra PSUM bank is needed.
            if warm_mms:
                wsrc = cpool.tile([P, 512], F16)
                wwt = cpool.tile([P, P], F16)
                nc.gpsimd.memset(wsrc[:], 0.25)
                nc.gpsimd.memset(wwt[:], 0.25)
                pw = ppool.tile([P, 512], F32, tag="pza", bufs=psum_bufs[2])
                for i in range(warm_mms):
                    nc.tensor.matmul(pw[:], wwt[:], wsrc[:],
                                     start=(i == 0), stop=(i == warm_mms - 1))

            # hoist ALL input DMAs up front on the SP ring (pure input
            # stream; pool slot-waits throttle the run-ahead)
            xts = []
            for r in range(ROUNDS):
                S = S_list[r]
                PR = P_LIST[r]
                xt = xpool.tile([P, 3 * S], F16, tag="xt")
                nc.sync.dma_start(
                    out=xt[:PR].rearrange("p (c w) -> p c w", c=3),
                    in_=xp.ap()[xbases[r] : xbases[r] + 3 * PR * S]
                    .rearrange("(c p s) -> p c s", c=3, p=PR),
                )
                xts.append(xt)

            emap = {"pool": "gpsimd", "vector": "vector", "gpsimd": "gpsimd",
                    "sync": "sync", "scalar": "scalar"}
            deng = getattr(nc, emap[diag_engine])
            veng = getattr(nc, emap.get(v_engine, "gpsimd"))
            oeng = getattr(nc, emap[out_engine])

            diags = {}

            def emit_diag(r, split_z=False):
                diag = dpool.tile([P, 8 * P], F16, tag="diag", name=f"diag{r}")
                if split_z:
                    # z diag blocks (6,7) first: unblocks round-0 z matmuls
                    deng.affine_select(
                        out=diag[:, 6 * P :].rearrange("p (k i) -> p k i",
                                                       k=2),
                        in_=ctile[:, r * 16 + 6 : r * 16 + 8]
                        .to_broadcast([P, 2, P]),
                        pattern=[[0, 2], [-1, P]],
                        channel_multiplier=1, base=0,
                        compare_op=mybir.AluOpType.is_equal, fill=0.0,
                    )
                    deng.affine_select(
                        out=diag[:, : 6 * P].rearrange("p (k i) -> p k i",
                                                       k=6),
                        in_=ctile[:, r * 16 : r * 16 + 6]
                        .to_broadcast([P, 6, P]),
                        pattern=[[0, 6], [-1, P]],
                        channel_multiplier=1, base=0,
                        compare_op=mybir.AluOpType.is_equal, fill=0.0,
                    )
                else:
                    deng.affine_select(
                        out=diag[:].rearrange("p (k i) -> p k i", k=8),
                        in_=ctile[:, r * 16 : r * 16 + 8]
                        .to_broadcast([P, 8, P]),
                        pattern=[[0, 8], [-1, P]],
                        channel_multiplier=1,
                        base=0,
                        compare_op=mybir.AluOpType.is_equal,
                        fill=0.0,
                    )
                diags[r] = diag

            # build the first diag_ahead rounds' weights before the loop;
            # diag r+diag_ahead is emitted inside round r (ahead of the Pool
            # v-mult) so Pool's in-order queue never serializes the pipe.
            for r in range(min(diag_ahead, ROUNDS)):
                emit_diag(r, split_z=(split_z0 and r == 0))

            # chunk list: (round, a, b) column spans; first/last rounds
            # optionally halved for shorter pipeline fill/drain
            chunks = []
            for r in range(ROUNDS):
                S = S_list[r]
                if r == 0 and head_halves:
                    h = (S // 2 + 3) & ~3
                    chunks.extend([(r, 0, h), (r, h, S)])
                elif r == ROUNDS - 1 and tail_halves:
                    h = (S // 2 + 3) & ~3
                    chunks.extend([(r, 0, h), (r, h, S)])
                else:
                    chunks.append((r, 0, S))

            q_tiles = {}

            def mk_q(cj):
                # one psum tile PER accumulation group: the ACT-init +
                # start=False accumulate breaks on hardware when two matmul
                # groups target one tile, so wide chunks get (pza, pzb).
                rj, aj, bj = chunks[cj]
                Wj = bj - aj
                x2j = xts[rj][:, 2 * S_list[rj] + aj : 2 * S_list[rj] + bj]
                scl = ctile[:, rj * 16 + 10 : rj * 16 + 11]
                bia = ctile[:, rj * 16 + 11 : rj * 16 + 12]
                Wa = min(Wj, 512)
                pza = ppool.tile([P, Wa], F32, tag="pza", bufs=psum_bufs[2],
                                 name=f"pza{cj}")
                nc.scalar.activation(out=pza[:], in_=x2j[:, 0:Wa], func=ident,
                                     scale=scl, bias=bia)
                pzb = None
                if Wj > 512:
                    pzb = ppool.tile([P, Wj - 512], F32, tag="pzb",
                                     bufs=psum_bufs[2], name=f"pzb{cj}")
                    nc.scalar.activation(out=pzb[:], in_=x2j[:, 512:Wj],
                                         func=ident, scale=scl, bias=bia)
                return pza, pzb

            for ci, (r, ca, cb) in enumerate(chunks):
                S = S_list[r]
                PR = P_LIST[r]
                W = cb - ca
                xt = xts[r]
                x0 = xt[:, 0 * S + ca : 0 * S + cb]
                x1 = xt[:, 1 * S + ca : 1 * S + cb]
                x2 = xt[:, 2 * S + ca : 2 * S + cb]

                def sc(j, r=r):
                    return ctile[:, r * 16 + 8 + j : r * 16 + 8 + j + 1]

                diag = diags[r]

                def dg(k):
                    return diag[:, k * P : (k + 1) * P]

                pun = ppool.tile([P, W], F32, tag="pun", bufs=psum_bufs[0])
                pvn = ppool.tile([P, W], F32, tag="pvn", bufs=psum_bufs[1])

                # z psum is initialized by ACT with q = c10*x2 + c11; the
                # two z matmuls then accumulate on top (start=False). The
                # init for chunk ci was emitted during chunk ci-1 (q_tiles).
                if ci == 0 or not q_ahead:
                    pza, pzb = mk_q(ci)
                else:
                    pza, pzb = q_tiles.pop(ci)

                splits = [(0, W)] if W <= 512 else [(0, 512), (512, W)]
                groups = {
                    "v": (pvn, [dg(3), dg(4), dg(5)], [x0, x1, x2], True),
                    "u": (pun, [dg(0), dg(1), dg(2)], [x0, x1, x2], True),
                }
                for key in mm_order:
                    if key == "z":
                        for pt, (a, b) in zip((pza, pzb), splits):
                            for k, dgk in enumerate([dg(6), dg(7)]):
                                nc.tensor.matmul(pt[:], dgk,
                                                 xt[:, k * S + ca + a :
                                                     k * S + ca + b],
                                                 start=False, stop=(k == 1))
                        continue
                    pt, dgs, xss, zero = groups[key]
                    for a, b in splits:
                        n = len(dgs)
                        for k in range(n):
                            nc.tensor.matmul(pt[:, a:b], dgs[k],
                                             xss[k][:, a:b],
                                             start=(zero and k == 0),
                                             stop=(k == n - 1))

                if ci + diag_ahead < len(chunks):
                    rn = chunks[ci + diag_ahead][0]
                    if rn not in diags:
                        emit_diag(rn)
                if q_ahead and ci + 1 < len(chunks):
                    q_tiles[ci + 1] = mk_q(ci + 1)

                rz = spool.tile([P, W], F16, tag="rz")
                with nc.allow_low_precision(reason="z in [5,15]; f16 rz"):
                    Wa = min(W, 512)
                    nc.vector.reciprocal(out=rz[:, 0:Wa], in_=pza[:])
                    if pzb is not None:
                        nc.vector.reciprocal(out=rz[:, 512:W], in_=pzb[:])

                uv = opool.tile([P, 2 * W], F16, tag="uv")
                last = ci == len(chunks) - 1
                if last and tail_split:
                    pass
                elif r in u_act_rounds:
                    # une = pun + c3 on ACT, u = une * rz on DVE (f16 2x tt)
                    une = spool.tile([P, W], F16, tag="une")
                    nc.scalar.activation(out=une[:], in_=pun[:], func=ident,
                                         bias=sc(0))
                    nc.vector.tensor_tensor(out=uv[:, 0:W], in0=une[:],
                                            in1=rz[:], op=mult)
                else:
                    # u = (pun + c3) * rz on DVE
                    nc.vector.scalar_tensor_tensor(out=uv[:, 0:W],
                                                   in0=pun[:], scalar=sc(0),
                                                   in1=rz[:], op0=add,
                                                   op1=mult)
                # (tail chunk handles u inside the tail_split block)
                ob = obases[r]
                oview = op.ap()[ob : ob + 2 * PR * S].rearrange(
                    "(c p s) -> c p s", c=2, p=PR)
                if last and tail_split:
                    # drain faster: u plane streams out while v computes
                    nc.vector.scalar_tensor_tensor(
                        out=uv[:, 0:W], in0=pun[:], scalar=sc(0),
                        in1=rz[:], op0=add, op1=mult)
                    oeng.dma_start(out=oview[0][:, ca:cb], in_=uv[:PR, 0:W])
                    nc.vector.scalar_tensor_tensor(
                        out=uv[:, W : 2 * W], in0=pvn[:], scalar=sc(1),
                        in1=rz[:], op0=add, op1=mult)
                    oeng.dma_start(out=oview[1][:, ca:cb],
                                   in_=uv[:PR, W : 2 * W])
                else:
                    if v_engine == "dve_stt":
                        nc.vector.scalar_tensor_tensor(
                            out=uv[:, W : 2 * W], in0=pvn[:], scalar=sc(1),
                            in1=rz[:], op0=add, op1=mult)
                    else:
                        # vn' = pvn + c7 on ACT; v = vn' * rz (f16 tt on DVE)
                        vne = spool.tile([P, W], F16, tag="vne")
                        nc.scalar.activation(out=vne[:], in_=pvn[:],
                                             func=ident, bias=sc(1))
                        if v_engine == "dve_tt":
                            nc.vector.tensor_tensor(out=uv[:, W : 2 * W],
                                                    in0=vne[:], in1=rz[:],
                                                    op=mult)
                        else:
                            veng.tensor_tensor(out=uv[:, W : 2 * W],
                                               in0=vne[:], in1=rz[:],
                                               op=mult)
                    oeng.dma_start(
                        out=oview[:, :, ca:cb].rearrange("c p s -> p c s"),
                        in_=uv[:PR].rearrange("p (c w) -> p c w", c=2),
                    )

    _legalize_single_wait(nc)
    return nc


def _plan(cam_all, B):
    """Per-core camera->slot layout + global round widths."""
    npts = B // NCORES
    plans = []
    S_dev_max = np.zeros(ROUNDS, np.int64)
    bounds = np.cumsum([0] + P_LIST)
    for d in range(NCORES):
        idx = cam_all[d * npts : (d + 1) * npts]
        cnt = np.bincount(idx, minlength=NUM_CAMERAS)
        order = np.argsort(-cnt, kind="stable")
        perm = np.argsort(idx, kind="stable")
        starts = np.zeros(NUM_CAMERAS, np.int64)
        starts[1:] = np.cumsum(cnt)[:-1]
        round_of = np.empty(NUM_CAMERAS, np.int64)
        part_of = np.empty(NUM_CAMERAS, np.int64)
        for r in range(ROUNDS):
            cams_r = order[bounds[r] : bounds[r + 1]]
            round_of[cams_r] = r
            part_of[cams_r] = np.arange(len(cams_r))
            S_dev_max[r] = max(S_dev_max[r], int(cnt[cams_r].max()))
        plans.append(dict(idx=idx, order=order, perm=perm, starts=starts,
                          round_of=round_of, part_of=part_of))
    S_list = [max(4, int(-(-s // 4) * 4)) for s in S_dev_max]
    return plans, S_list


def kernel(X_world, camera_indices, noisy_K, noisy_R, noisy_t,
           intrinsic_deltas, rotation_deltas, translation_deltas):
    global LAST_RESULTS, LAST_S_LIST

    X_world = np.asarray(X_world, dtype=np.float32)
    cam_all = np.asarray(camera_indices).astype(np.int64)
    B = X_world.shape[0]
    assert B % NCORES == 0
    npts = B // NCORES

    C = _camera_coeffs(
        np.asarray(noisy_K, np.float32), np.asarray(noisy_R, np.float32),
        np.asarray(noisy_t, np.float32),
        np.asarray(intrinsic_deltas, np.float32),
        np.asarray(rotation_deltas, np.float32),
        np.asarray(translation_deltas, np.float32),
    )

    plans, S_list = _plan(cam_all, B)
    LAST_S_LIST = S_list
    S_arr = np.asarray(S_list, np.int64)
    P_arr = np.asarray(P_LIST, np.int64)
    xbase = np.zeros(ROUNDS, np.int64)
    obase = np.zeros(ROUNDS, np.int64)
    for r in range(1, ROUNDS):
        xbase[r] = xbase[r - 1] + 3 * P_LIST[r - 1] * S_list[r - 1]
        obase[r] = obase[r - 1] + 2 * P_LIST[r - 1] * S_list[r - 1]
    totx = int(xbase[-1] + 3 * P_LIST[-1] * S_list[-1])
    toto = int(obase[-1] + 2 * P_LIST[-1] * S_list[-1])

    # ---- pack per-core inputs (f16 planes, camera-grouped) ----
    # coef column layout per round r (16 cols):
    #   [0:8]  mm coefficients (c0,c1,c2, c4,c5,c6, c8,c9) -> diag blocks
    #   [8:12] scalars (c3, c7, c10, c11)
    in_maps = []
    for d in range(NCORES):
        p = plans[d]
        Xd = X_world[d * npts : (d + 1) * npts]
        S_of = S_arr[p["round_of"]]
        plane_of = P_arr[p["round_of"]] * S_of
        xb0 = xbase[p["round_of"]] + p["part_of"] * S_of
        cam_sorted = p["idx"][p["perm"]]
        j_sorted = np.arange(npts, dtype=np.int64) - p["starts"][cam_sorted]
        a0 = xb0[cam_sorted] + j_sorted
        plane = plane_of[cam_sorted]
        xp = np.zeros(totx, np.float16)
        Xs = Xd[p["perm"]].astype(np.float16)
        xp[a0] = Xs[:, 0]
        xp[a0 + plane] = Xs[:, 1]
        xp[a0 + 2 * plane] = Xs[:, 2]

        coef_d = np.zeros((ROUNDS * P, 16), np.float32)
        coef_d[:, 11] = 1.0  # pad slots: q = 0*x2 + 1 -> z=1, no inf/nan
        slot_rows = p["round_of"] * P + p["part_of"]
        mm_cols = [0, 1, 2, 4, 5, 6, 8, 9]
        sc_cols = [3, 7, 10, 11]
        coef_d[np.ix_(slot_rows, np.arange(8))] = C[:, mm_cols]
        coef_d[np.ix_(slot_rows, np.arange(8, 12))] = C[:, sc_cols]
        in_maps.append({"xp": xp, "coef": coef_d})

        p["cam_sorted"] = cam_sorted
        p["j_sorted"] = j_sorted

    # ---- build + run on the 8 NeuronCores ----
    nc = _build_program(S_list, **BEST_CONFIG)
    try:
        res = run_bass_kernel_spmd(nc, in_maps, list(range(NCORES)))
    except Exception:
        res = run_bass_kernel_spmd(nc, in_maps, list(range(NCORES)))
    LAST_RESULTS = res

    # ---- unscatter back to original point order ----
    out = np.empty((B, 2), np.float32)
    for d in range(NCORES):
        p = plans[d]
        S_of = S_arr[p["round_of"]]
        ub = obase[p["round_of"]] + p["part_of"] * S_of
        ua = ub[p["cam_sorted"]] + p["j_sorted"]
        va = ua + (P_arr[p["round_of"]] * S_of)[p["cam_sorted"]]
        opd = res.results[d]["op"].astype(np.float32)
        od = out[d * npts : (d + 1) * npts]
        od[p["perm"], 0] = opd[ua]
        od[p["perm"], 1] = opd[va]
    return out
